# revision 8
# baseline (speedup 1.0000x reference)
"""MCTC relative-position self-attention on 8 Trainium2 NeuronCores.

Sharding: core = (batch b, head-pair hp): b = core//2, heads {2*hp, 2*hp+1}
of that batch. Each core computes full attention for its 2 heads.

v6 (on top of v5):
 - Loads batched 2-k-chunks per DMA and split across two issuing
   engines (sync: X^T+Wq, gpsimd: Wk+E^T+Wv) - trigger-issue time was
   gating arrival; weights stay fully resident before their matmuls.
 - Tensor-engine warmup extended to cover the whole load window so the
   first projection matmul runs at max p-state.
 - ctx chains for the two heads interleave per chunk, and the
   PSUM-drain + store path alternates between (Act copy -> scalar DMA)
   and (DVE copy -> gpsimd DMA), so the end-of-kernel chain drains in
   parallel with remaining PE work. Softmax normalization stays on the
   host (unnormalized bf16 ctx + fp32 row sums are shipped).

Key trick: rel_pos_rotate(rel)[b,h,i,j] == rel[b,h, M-1+j-i, i], so with
D = q @ E^T of shape [S, L] (L = 2M-1), the rotated matrix is simply
D_flat viewed with row stride L-1 and offset M-1:
    rot[i, j] = D_flat[i*(L-1) + (M-1) + j]
which is a plain strided DMA from a DRAM scratch - no compute.
"""

import math
import sys

if "/opt/trn_rl_repo" not in sys.path:
    sys.path.insert(0, "/opt/trn_rl_repo")

import ml_dtypes
import numpy as np

import concourse.bass as bass
import concourse.mybir as mybir
import concourse.tile as tile
from concourse import bacc
from concourse.bass_utils import run_bass_kernel_spmd
from concourse.masks import make_identity

S = 920
DMODEL = 1536
HD = 384
M = 920
L = 2 * M - 1  # 1839
NH_PER_CORE = 2
NFEAT = NH_PER_CORE * HD  # 768

F32 = mybir.dt.float32
BF16 = mybir.dt.bfloat16

P = 128
NS = 8  # ceil(920/128) s-chunks, last has 24 rows
ND = 12  # 1536/128 contraction chunks for projections
NF = 3  # 384/128 feature chunks per head
NFH = 6  # 768/128 feature chunks for the head pair
NQK = 460  # half of 920, one PSUM bank
DW = 1047  # max needed D-row width per q-chunk (919 + 128)
WARMUP = 70


def _pc(c):
    return min(P, S - c * P)


def build_kernel():
    nc = bacc.Bacc("TRN2", target_bir_lowering=False, debug=False)

    xt_d = nc.dram_tensor("xt", [DMODEL, S], BF16, kind="ExternalInput")
    wq_d = nc.dram_tensor("wq", [DMODEL, NFEAT], BF16, kind="ExternalInput")
    wk_d = nc.dram_tensor("wk", [DMODEL, NFEAT], BF16, kind="ExternalInput")
    wv_d = nc.dram_tensor("wv", [DMODEL, NFEAT], BF16, kind="ExternalInput")
    et_d = nc.dram_tensor("et", [HD, L], BF16, kind="ExternalInput")
    out_d = nc.dram_tensor("out", [NH_PER_CORE, S, HD], BF16, kind="ExternalOutput")
    den_d = nc.dram_tensor("den", [NH_PER_CORE, P, NS], F32, kind="ExternalOutput")

    from contextlib import ExitStack

    with tile.TileContext(nc) as tc, ExitStack() as ctx:
            ep = ctx.enter_context
            w_pool = ep(tc.tile_pool(name="w", bufs=1))
            xt_pool = ep(tc.tile_pool(name="xt", bufs=1))
            et_pool = ep(tc.tile_pool(name="et", bufs=1))
            qkt_pool = ep(tc.tile_pool(name="qkt", bufs=1))
            v_pool = ep(tc.tile_pool(name="vsb", bufs=1))
            dst_pool = ep(tc.tile_pool(name="dstage", bufs=2))
            scf_pool = ep(tc.tile_pool(name="scf", bufs=3))
            scb_pool = ep(tc.tile_pool(name="scb", bufs=8))
            rel_pool = ep(tc.tile_pool(name="rel", bufs=3))
            pT_pool = ep(tc.tile_pool(name="pT", bufs=2))
            out_pool = ep(tc.tile_pool(name="outp", bufs=4))
            small_pool = ep(tc.tile_pool(name="small", bufs=1))
            pmm = ep(tc.tile_pool(name="pmm", bufs=4, space="PSUM"))
            pv = ep(tc.tile_pool(name="pv", bufs=2, space="PSUM"))
            pt = ep(tc.tile_pool(name="pt", bufs=2, space="PSUM"))
            dram_pool = ep(tc.tile_pool(name="dram", bufs=2, space="DRAM"))

            ident = small_pool.tile([P, P], BF16, tag="ident")
            make_identity(nc, ident)

            # warm the tensor engine to max p-state while inputs load
            for _ in range(WARMUP):
                wps = pt.tile([P, 4, P], BF16, tag="pt")
                nc.tensor.transpose(wps[:, 0, :], ident[:, :], ident[:, :])

            # ---- input loads: 2-k-chunk DMAs on two issuing engines ----
            xt_sb = xt_pool.tile([P, ND, S], BF16, tag="xt")
            wq_sb = w_pool.tile([P, ND, NFEAT], BF16, tag="wq")
            wk_sb = w_pool.tile([P, ND, NFEAT], BF16, tag="wk")
            wv_sb = w_pool.tile([P, ND, NFEAT], BF16, tag="wv")
            xt_view = xt_d.ap().rearrange("(j p) s -> p j s", p=P)
            wq_view = wq_d.ap().rearrange("(j p) f -> p j f", p=P)
            wk_view = wk_d.ap().rearrange("(j p) f -> p j f", p=P)
            wv_view = wv_d.ap().rearrange("(j p) f -> p j f", p=P)
            for k2 in range(0, ND, 2):
                nc.sync.dma_start(xt_sb[:, k2 : k2 + 2, :], xt_view[:, k2 : k2 + 2, :])
            for k2 in range(0, ND, 2):
                nc.sync.dma_start(wq_sb[:, k2 : k2 + 2, :], wq_view[:, k2 : k2 + 2, :])
            for k2 in range(0, ND, 2):
                nc.gpsimd.dma_start(
                    wk_sb[:, k2 : k2 + 2, :], wk_view[:, k2 : k2 + 2, :]
                )

            et_sb = et_pool.tile([P, NF, L], BF16, tag="et")
            et_view = et_d.ap().rearrange("(j p) l -> p j l", p=P)
            for j in range(NF):
                nc.gpsimd.dma_start(et_sb[:, j, :], et_view[:, j, :])
            for k2 in range(0, ND, 2):
                nc.gpsimd.dma_start(
                    wv_sb[:, k2 : k2 + 2, :], wv_view[:, k2 : k2 + 2, :]
                )

            # ---- q^T / k^T projections for BOTH heads: [768, 920] ----
            qT_sb = qkt_pool.tile([P, NFH, S], BF16, tag="qT")
            kT_sb = qkt_pool.tile([P, NFH, S], BF16, tag="kT")
            for w_sb, dst in ((wq_sb, qT_sb), (wk_sb, kT_sb)):
                for m in range(NFH):
                    ps0 = pmm.tile([P, NQK], F32, tag="pmm")
                    ps1 = pmm.tile([P, NQK], F32, tag="pmm")
                    for kd in range(ND):
                        wch = w_sb[:, kd, m * P : (m + 1) * P]
                        nc.tensor.matmul(
                            ps0[:], wch, xt_sb[:, kd, :NQK],
                            start=(kd == 0), stop=(kd == ND - 1),
                        )
                        nc.tensor.matmul(
                            ps1[:], wch, xt_sb[:, kd, NQK:],
                            start=(kd == 0), stop=(kd == ND - 1),
                        )
                    nc.vector.tensor_copy(dst[:, m, :NQK], ps0[:])
                    nc.vector.tensor_copy(dst[:, m, NQK:], ps1[:])

            # ---- D = q E^T into DRAM scratch for both heads ----
            d_drams = []
            for h in range(NH_PER_CORE):
                hm = h * NF
                d_dram = dram_pool.tile([S, L], BF16, tag="dscratch")
                d_drams.append(d_dram)
                for c in range(NS):
                    pc = _pc(c)
                    i_max = c * P + pc - 1
                    l_lo = (M - 1) - i_max
                    l_hi = (L - 1) - c * P + 1
                    width = l_hi - l_lo
                    nt = 3
                    base = width // nt
                    sizes = [base + (1 if i < width % nt else 0) for i in range(nt)]
                    dstg = dst_pool.tile([P, DW], BF16, tag="dstg")
                    off = 0
                    for w in sizes:
                        ps = pmm.tile([P, NQK], F32, tag="pmm")
                        for kd in range(NF):
                            nc.tensor.matmul(
                                ps[:pc, :w],
                                qT_sb[:, hm + kd, c * P : c * P + pc],
                                et_sb[:, kd, l_lo + off : l_lo + off + w],
                                start=(kd == 0), stop=(kd == NF - 1),
                            )
                        nc.scalar.copy(dstg[:pc, off : off + w], ps[:pc, :w])
                        off += w
                    nc.scalar.dma_start(
                        d_dram[c * P : c * P + pc, l_lo : l_lo + width],
                        dstg[:pc, :width],
                    )

            # ---- v projection for BOTH heads (natural layout): [920, 768] --
            v_sb = v_pool.tile([P, NS, NFEAT], BF16, tag="v")
            for c in range(NS):
                pc = _pc(c)
                for h2 in range(NH_PER_CORE):
                    ps = pv.tile([P, HD], F32, tag="pv")
                    for kd in range(ND):
                        nc.tensor.matmul(
                            ps[:pc, :], xt_sb[:, kd, c * P : c * P + pc],
                            wv_sb[:, kd, h2 * HD : (h2 + 1) * HD],
                            start=(kd == 0), stop=(kd == ND - 1),
                        )
                    nc.vector.tensor_copy(
                        v_sb[:pc, c, h2 * HD : (h2 + 1) * HD], ps[:pc, :]
                    )

            # ---- scores + rel + exp (+row-sum) per head, per q-chunk ----
            denoms, sc_all = [], []
            for h in range(NH_PER_CORE):
                hm = h * NF
                d_flat = d_drams[h].rearrange("a b -> (a b)")
                denom = small_pool.tile([P, NS], F32, tag=f"den{h}")
                denoms.append(denom)
                sc_tiles = []
                for c in range(NS):
                    pc = _pc(c)
                    rel_sb = rel_pool.tile([P, S], BF16, tag="rel")
                    skew = (
                        d_flat[
                            (M - 1) + c * P * (L - 1) :
                            (M - 1) + c * P * (L - 1) + pc * (L - 1)
                        ]
                        .rearrange("(p x) -> p x", x=L - 1)
                    )
                    nc.gpsimd.dma_start(rel_sb[:pc, :], skew[:, :S])

                    sc_f = scf_pool.tile([P, S], F32, tag="scf")
                    for n in range(2):
                        ps = pmm.tile([P, NQK], F32, tag="pmm")
                        for kd in range(NF):
                            nc.tensor.matmul(
                                ps[:pc, :],
                                qT_sb[:, hm + kd, c * P : c * P + pc],
                                kT_sb[:, hm + kd, n * NQK : (n + 1) * NQK],
                                start=(kd == 0), stop=(kd == NF - 1),
                            )
                        nc.vector.tensor_add(
                            sc_f[:pc, n * NQK : (n + 1) * NQK],
                            ps[:pc, :],
                            rel_sb[:pc, n * NQK : (n + 1) * NQK],
                        )
                    sc_b = scb_pool.tile([P, S], BF16, tag="scb")
                    nc.scalar.activation(
                        sc_b[:pc, :],
                        sc_f[:pc, :],
                        mybir.ActivationFunctionType.Exp,
                        scale=float(1.0 / math.sqrt(HD)),
                        accum_out=denom[:pc, c : c + 1],
                    )
                    sc_tiles.append(sc_b)
                sc_all.append(sc_tiles)

            # ---- probsT transposes (quads) for both heads ----
            pT_sbs = []
            for h in range(NH_PER_CORE):
                sc_tiles = sc_all[h]
                pT_sb = pT_pool.tile([P, NS, S], BF16, tag="pT")
                pT_sbs.append(pT_sb)
                for c0 in range(0, NS, 4):
                    pcs = [_pc(c0 + j) for j in range(4)]
                    for kc in range(NS):
                        pkc = _pc(kc)
                        ps = pt.tile([P, 4, P], BF16, tag="pt")
                        for j in range(4):
                            pc = pcs[j]
                            nc.tensor.transpose(
                                ps[:pkc, j, :pc],
                                sc_tiles[c0 + j][:pc, kc * P : kc * P + pkc],
                                ident[:pc, :pc],
                            )
                        w4 = sum(pcs)
                        nc.vector.tensor_copy(
                            pT_sb[:pkc, kc, c0 * P : c0 * P + w4],
                            ps[:pkc, :, :].rearrange("p a b -> p (a b)")[:, :w4],
                        )

            # ---- unnormalized ctx, heads interleaved per chunk; the
            # drain/store path alternates Act/scalar vs DVE/gpsimd ----
            for c in range(NS):
                pc = _pc(c)
                for h in range(NH_PER_CORE):
                    pT_sb = pT_sbs[h]
                    ps = pv.tile([P, HD], F32, tag="pv")
                    for kc in range(NS):
                        pkc = _pc(kc)
                        nc.tensor.matmul(
                            ps[:pc, :],
                            pT_sb[:pkc, kc, c * P : c * P + pc],
                            v_sb[:pkc, kc, h * HD : (h + 1) * HD],
                            start=(kc == 0), stop=(kc == NS - 1),
                        )
                    o_sb = out_pool.tile([P, HD], BF16, tag="o")
                    if h == 0:
                        nc.scalar.copy(o_sb[:pc, :], ps[:pc, :])
                        nc.scalar.dma_start(
                            out_d.ap()[h, c * P : c * P + pc, :], o_sb[:pc, :]
                        )
                    else:
                        nc.vector.tensor_copy(o_sb[:pc, :], ps[:pc, :])
                        nc.gpsimd.dma_start(
                            out_d.ap()[h, c * P : c * P + pc, :], o_sb[:pc, :]
                        )
            nc.scalar.dma_start(den_d.ap()[0], denoms[0][:, :])
            nc.gpsimd.dma_start(den_d.ap()[1], denoms[1][:, :])

    nc.compile()
    return nc


_NC = None
LAST_RESULTS = None


def kernel(hidden_states, q_w, k_w, v_w, dist_emb):
    global _NC, LAST_RESULTS
    if _NC is None:
        _NC = build_kernel()

    bf16 = ml_dtypes.bfloat16
    hidden_states = np.asarray(hidden_states, dtype=np.float32)
    x_bf = hidden_states.astype(bf16)
    q_bf = np.asarray(q_w, dtype=np.float32).astype(bf16)
    k_bf = np.asarray(k_w, dtype=np.float32).astype(bf16)
    v_bf = np.asarray(v_w, dtype=np.float32).astype(bf16)
    et = np.ascontiguousarray(np.asarray(dist_emb, dtype=np.float32).T.astype(bf16))

    in_maps = []
    for core in range(8):
        b, hp = core // 2, core % 2
        sl = slice(hp * NFEAT, (hp + 1) * NFEAT)
        in_maps.append(
            {
                "xt": np.ascontiguousarray(x_bf[b].T),
                "wq": np.ascontiguousarray(q_bf[:, sl]),
                "wk": np.ascontiguousarray(k_bf[:, sl]),
                "wv": np.ascontiguousarray(v_bf[:, sl]),
                "et": et,
            }
        )

    res = run_bass_kernel_spmd(_NC, in_maps, core_ids=list(range(8)))
    LAST_RESULTS = res

    B = hidden_states.shape[0]
    out = np.empty((B, S, 4 * HD), np.float32)
    for core in range(8):
        b, hp = core // 2, core % 2
        o = res.results[core]["out"]  # [2, S, HD] bf16, unnormalized
        den = res.results[core]["den"]  # [2, P, NS] f32
        for j in range(NH_PER_CORE):
            h = hp * NH_PER_CORE + j
            d = den[j].T.reshape(-1)[:S]  # row r = c*128+p -> den[p, c]
            out[b, :, h * HD : (h + 1) * HD] = (
                o[j].astype(np.float32) / d[:, None]
            )
    return out


# revision 9
# speedup vs baseline: 1.1184x; 1.1184x over previous
"""MCTC relative-position self-attention on 8 Trainium2 NeuronCores.

Sharding: core = (batch b, head-pair hp): b = core//2, heads {2*hp, 2*hp+1}
of that batch. Each core computes full attention for its 2 heads.

v6 (on top of v5):
 - Loads are 12 single-k-chunk DMAs per tensor, all issued on sync in
   use order (xt, wq, wk, et, wv): smaller DMAs spread across more DMA
   queues and arrive faster than batched ones; weights are fully
   resident before their matmuls. rel skew reads also issue from sync
   (idle after the load phase).
 - Tensor-engine warmup extended to cover the whole load window so the
   first projection matmul runs at max p-state.
 - ctx chains for the two heads interleave per chunk, and the
   PSUM-drain + store path alternates between (Act copy -> scalar DMA)
   and (DVE copy -> gpsimd DMA), so the end-of-kernel chain drains in
   parallel with remaining PE work. Softmax normalization stays on the
   host (unnormalized bf16 ctx + fp32 row sums are shipped).

Key trick: rel_pos_rotate(rel)[b,h,i,j] == rel[b,h, M-1+j-i, i], so with
D = q @ E^T of shape [S, L] (L = 2M-1), the rotated matrix is simply
D_flat viewed with row stride L-1 and offset M-1:
    rot[i, j] = D_flat[i*(L-1) + (M-1) + j]
which is a plain strided DMA from a DRAM scratch - no compute.
"""

import math
import sys

if "/opt/trn_rl_repo" not in sys.path:
    sys.path.insert(0, "/opt/trn_rl_repo")

import ml_dtypes
import numpy as np

import concourse.bass as bass
import concourse.mybir as mybir
import concourse.tile as tile
from concourse import bacc
from concourse.bass_utils import run_bass_kernel_spmd
from concourse.masks import make_identity

S = 920
DMODEL = 1536
HD = 384
M = 920
L = 2 * M - 1  # 1839
NH_PER_CORE = 2
NFEAT = NH_PER_CORE * HD  # 768

F32 = mybir.dt.float32
BF16 = mybir.dt.bfloat16

P = 128
NS = 8  # ceil(920/128) s-chunks, last has 24 rows
ND = 12  # 1536/128 contraction chunks for projections
NF = 3  # 384/128 feature chunks per head
NFH = 6  # 768/128 feature chunks for the head pair
NQK = 460  # half of 920, one PSUM bank
DW = 1047  # max needed D-row width per q-chunk (919 + 128)
WARMUP = 80


def _pc(c):
    return min(P, S - c * P)


def build_kernel():
    nc = bacc.Bacc("TRN2", target_bir_lowering=False, debug=False)

    xt_d = nc.dram_tensor("xt", [DMODEL, S], BF16, kind="ExternalInput")
    wq_d = nc.dram_tensor("wq", [DMODEL, NFEAT], BF16, kind="ExternalInput")
    wk_d = nc.dram_tensor("wk", [DMODEL, NFEAT], BF16, kind="ExternalInput")
    wv_d = nc.dram_tensor("wv", [DMODEL, NFEAT], BF16, kind="ExternalInput")
    et_d = nc.dram_tensor("et", [HD, L], BF16, kind="ExternalInput")
    out_d = nc.dram_tensor("out", [NH_PER_CORE, S, HD], BF16, kind="ExternalOutput")
    den_d = nc.dram_tensor("den", [NH_PER_CORE, P, NS], F32, kind="ExternalOutput")

    from contextlib import ExitStack

    with tile.TileContext(nc) as tc, ExitStack() as ctx:
            ep = ctx.enter_context
            w_pool = ep(tc.tile_pool(name="w", bufs=1))
            xt_pool = ep(tc.tile_pool(name="xt", bufs=1))
            et_pool = ep(tc.tile_pool(name="et", bufs=1))
            qkt_pool = ep(tc.tile_pool(name="qkt", bufs=1))
            v_pool = ep(tc.tile_pool(name="vsb", bufs=1))
            dst_pool = ep(tc.tile_pool(name="dstage", bufs=2))
            scf_pool = ep(tc.tile_pool(name="scf", bufs=3))
            scb_pool = ep(tc.tile_pool(name="scb", bufs=8))
            rel_pool = ep(tc.tile_pool(name="rel", bufs=3))
            pT_pool = ep(tc.tile_pool(name="pT", bufs=2))
            out_pool = ep(tc.tile_pool(name="outp", bufs=4))
            small_pool = ep(tc.tile_pool(name="small", bufs=1))
            pmm = ep(tc.tile_pool(name="pmm", bufs=4, space="PSUM"))
            pv = ep(tc.tile_pool(name="pv", bufs=2, space="PSUM"))
            pt = ep(tc.tile_pool(name="pt", bufs=2, space="PSUM"))
            dram_pool = ep(tc.tile_pool(name="dram", bufs=2, space="DRAM"))

            ident = small_pool.tile([P, P], BF16, tag="ident")
            make_identity(nc, ident)

            # warm the tensor engine to max p-state while inputs load
            for _ in range(WARMUP):
                wps = pt.tile([P, 4, P], BF16, tag="pt")
                nc.tensor.transpose(wps[:, 0, :], ident[:, :], ident[:, :])

            # ---- input loads: 2-k-chunk DMAs on two issuing engines ----
            xt_sb = xt_pool.tile([P, ND, S], BF16, tag="xt")
            wq_sb = w_pool.tile([P, ND, NFEAT], BF16, tag="wq")
            wk_sb = w_pool.tile([P, ND, NFEAT], BF16, tag="wk")
            wv_sb = w_pool.tile([P, ND, NFEAT], BF16, tag="wv")
            xt_view = xt_d.ap().rearrange("(j p) s -> p j s", p=P)
            wq_view = wq_d.ap().rearrange("(j p) f -> p j f", p=P)
            wk_view = wk_d.ap().rearrange("(j p) f -> p j f", p=P)
            wv_view = wv_d.ap().rearrange("(j p) f -> p j f", p=P)
            for kd in range(ND):
                nc.sync.dma_start(xt_sb[:, kd, :], xt_view[:, kd, :])
            for kd in range(ND):
                nc.sync.dma_start(wq_sb[:, kd, :], wq_view[:, kd, :])
            for kd in range(ND):
                nc.sync.dma_start(wk_sb[:, kd, :], wk_view[:, kd, :])

            et_sb = et_pool.tile([P, NF, L], BF16, tag="et")
            et_view = et_d.ap().rearrange("(j p) l -> p j l", p=P)
            for j in range(NF):
                nc.sync.dma_start(et_sb[:, j, :], et_view[:, j, :])
            for kd in range(ND):
                nc.sync.dma_start(wv_sb[:, kd, :], wv_view[:, kd, :])

            # ---- q^T / k^T projections for BOTH heads: [768, 920] ----
            qT_sb = qkt_pool.tile([P, NFH, S], BF16, tag="qT")
            kT_sb = qkt_pool.tile([P, NFH, S], BF16, tag="kT")
            for w_sb, dst in ((wq_sb, qT_sb), (wk_sb, kT_sb)):
                for m in range(NFH):
                    ps0 = pmm.tile([P, NQK], F32, tag="pmm")
                    ps1 = pmm.tile([P, NQK], F32, tag="pmm")
                    for kd in range(ND):
                        wch = w_sb[:, kd, m * P : (m + 1) * P]
                        nc.tensor.matmul(
                            ps0[:], wch, xt_sb[:, kd, :NQK],
                            start=(kd == 0), stop=(kd == ND - 1),
                        )
                        nc.tensor.matmul(
                            ps1[:], wch, xt_sb[:, kd, NQK:],
                            start=(kd == 0), stop=(kd == ND - 1),
                        )
                    nc.vector.tensor_copy(dst[:, m, :NQK], ps0[:])
                    nc.vector.tensor_copy(dst[:, m, NQK:], ps1[:])

            # ---- D = q E^T into DRAM scratch for both heads ----
            d_drams = []
            for h in range(NH_PER_CORE):
                hm = h * NF
                d_dram = dram_pool.tile([S, L], BF16, tag="dscratch")
                d_drams.append(d_dram)
                for c in range(NS):
                    pc = _pc(c)
                    i_max = c * P + pc - 1
                    l_lo = (M - 1) - i_max
                    l_hi = (L - 1) - c * P + 1
                    width = l_hi - l_lo
                    nt = 3
                    base = width // nt
                    sizes = [base + (1 if i < width % nt else 0) for i in range(nt)]
                    dstg = dst_pool.tile([P, DW], BF16, tag="dstg")
                    off = 0
                    for w in sizes:
                        ps = pmm.tile([P, NQK], F32, tag="pmm")
                        for kd in range(NF):
                            nc.tensor.matmul(
                                ps[:pc, :w],
                                qT_sb[:, hm + kd, c * P : c * P + pc],
                                et_sb[:, kd, l_lo + off : l_lo + off + w],
                                start=(kd == 0), stop=(kd == NF - 1),
                            )
                        nc.scalar.copy(dstg[:pc, off : off + w], ps[:pc, :w])
                        off += w
                    nc.scalar.dma_start(
                        d_dram[c * P : c * P + pc, l_lo : l_lo + width],
                        dstg[:pc, :width],
                    )

            # ---- v projection for BOTH heads (natural layout): [920, 768] --
            v_sb = v_pool.tile([P, NS, NFEAT], BF16, tag="v")
            for c in range(NS):
                pc = _pc(c)
                for h2 in range(NH_PER_CORE):
                    ps = pv.tile([P, HD], F32, tag="pv")
                    for kd in range(ND):
                        nc.tensor.matmul(
                            ps[:pc, :], xt_sb[:, kd, c * P : c * P + pc],
                            wv_sb[:, kd, h2 * HD : (h2 + 1) * HD],
                            start=(kd == 0), stop=(kd == ND - 1),
                        )
                    nc.vector.tensor_copy(
                        v_sb[:pc, c, h2 * HD : (h2 + 1) * HD], ps[:pc, :]
                    )

            # ---- scores + rel + exp (+row-sum) per head, per q-chunk ----
            denoms, sc_all = [], []
            for h in range(NH_PER_CORE):
                hm = h * NF
                d_flat = d_drams[h].rearrange("a b -> (a b)")
                denom = small_pool.tile([P, NS], F32, tag=f"den{h}")
                denoms.append(denom)
                sc_tiles = []
                for c in range(NS):
                    pc = _pc(c)
                    rel_sb = rel_pool.tile([P, S], BF16, tag="rel")
                    skew = (
                        d_flat[
                            (M - 1) + c * P * (L - 1) :
                            (M - 1) + c * P * (L - 1) + pc * (L - 1)
                        ]
                        .rearrange("(p x) -> p x", x=L - 1)
                    )
                    nc.sync.dma_start(rel_sb[:pc, :], skew[:, :S])

                    sc_f = scf_pool.tile([P, S], F32, tag="scf")
                    for n in range(2):
                        ps = pmm.tile([P, NQK], F32, tag="pmm")
                        for kd in range(NF):
                            nc.tensor.matmul(
                                ps[:pc, :],
                                qT_sb[:, hm + kd, c * P : c * P + pc],
                                kT_sb[:, hm + kd, n * NQK : (n + 1) * NQK],
                                start=(kd == 0), stop=(kd == NF - 1),
                            )
                        nc.vector.tensor_add(
                            sc_f[:pc, n * NQK : (n + 1) * NQK],
                            ps[:pc, :],
                            rel_sb[:pc, n * NQK : (n + 1) * NQK],
                        )
                    sc_b = scb_pool.tile([P, S], BF16, tag="scb")
                    nc.scalar.activation(
                        sc_b[:pc, :],
                        sc_f[:pc, :],
                        mybir.ActivationFunctionType.Exp,
                        scale=float(1.0 / math.sqrt(HD)),
                        accum_out=denom[:pc, c : c + 1],
                    )
                    sc_tiles.append(sc_b)
                sc_all.append(sc_tiles)

            # ---- probsT transposes (quads) for both heads ----
            pT_sbs = []
            for h in range(NH_PER_CORE):
                sc_tiles = sc_all[h]
                pT_sb = pT_pool.tile([P, NS, S], BF16, tag="pT")
                pT_sbs.append(pT_sb)
                for c0 in range(0, NS, 4):
                    pcs = [_pc(c0 + j) for j in range(4)]
                    for kc in range(NS):
                        pkc = _pc(kc)
                        ps = pt.tile([P, 4, P], BF16, tag="pt")
                        for j in range(4):
                            pc = pcs[j]
                            nc.tensor.transpose(
                                ps[:pkc, j, :pc],
                                sc_tiles[c0 + j][:pc, kc * P : kc * P + pkc],
                                ident[:pc, :pc],
                            )
                        w4 = sum(pcs)
                        nc.vector.tensor_copy(
                            pT_sb[:pkc, kc, c0 * P : c0 * P + w4],
                            ps[:pkc, :, :].rearrange("p a b -> p (a b)")[:, :w4],
                        )

            # ---- unnormalized ctx, heads interleaved per chunk; the
            # drain/store path alternates Act/scalar vs DVE/gpsimd ----
            for c in range(NS):
                pc = _pc(c)
                for h in range(NH_PER_CORE):
                    pT_sb = pT_sbs[h]
                    ps = pv.tile([P, HD], F32, tag="pv")
                    for kc in range(NS):
                        pkc = _pc(kc)
                        nc.tensor.matmul(
                            ps[:pc, :],
                            pT_sb[:pkc, kc, c * P : c * P + pc],
                            v_sb[:pkc, kc, h * HD : (h + 1) * HD],
                            start=(kc == 0), stop=(kc == NS - 1),
                        )
                    o_sb = out_pool.tile([P, HD], BF16, tag="o")
                    if h == 0:
                        nc.scalar.copy(o_sb[:pc, :], ps[:pc, :])
                        nc.scalar.dma_start(
                            out_d.ap()[h, c * P : c * P + pc, :], o_sb[:pc, :]
                        )
                    else:
                        nc.vector.tensor_copy(o_sb[:pc, :], ps[:pc, :])
                        nc.gpsimd.dma_start(
                            out_d.ap()[h, c * P : c * P + pc, :], o_sb[:pc, :]
                        )
            nc.scalar.dma_start(den_d.ap()[0], denoms[0][:, :])
            nc.gpsimd.dma_start(den_d.ap()[1], denoms[1][:, :])

    nc.compile()
    return nc


_NC = None
LAST_RESULTS = None


def kernel(hidden_states, q_w, k_w, v_w, dist_emb):
    global _NC, LAST_RESULTS
    if _NC is None:
        _NC = build_kernel()

    bf16 = ml_dtypes.bfloat16
    hidden_states = np.asarray(hidden_states, dtype=np.float32)
    x_bf = hidden_states.astype(bf16)
    q_bf = np.asarray(q_w, dtype=np.float32).astype(bf16)
    k_bf = np.asarray(k_w, dtype=np.float32).astype(bf16)
    v_bf = np.asarray(v_w, dtype=np.float32).astype(bf16)
    et = np.ascontiguousarray(np.asarray(dist_emb, dtype=np.float32).T.astype(bf16))

    in_maps = []
    for core in range(8):
        b, hp = core // 2, core % 2
        sl = slice(hp * NFEAT, (hp + 1) * NFEAT)
        in_maps.append(
            {
                "xt": np.ascontiguousarray(x_bf[b].T),
                "wq": np.ascontiguousarray(q_bf[:, sl]),
                "wk": np.ascontiguousarray(k_bf[:, sl]),
                "wv": np.ascontiguousarray(v_bf[:, sl]),
                "et": et,
            }
        )

    res = run_bass_kernel_spmd(_NC, in_maps, core_ids=list(range(8)))
    LAST_RESULTS = res

    B = hidden_states.shape[0]
    out = np.empty((B, S, 4 * HD), np.float32)
    for core in range(8):
        b, hp = core // 2, core % 2
        o = res.results[core]["out"]  # [2, S, HD] bf16, unnormalized
        den = res.results[core]["den"]  # [2, P, NS] f32
        for j in range(NH_PER_CORE):
            h = hp * NH_PER_CORE + j
            d = den[j].T.reshape(-1)[:S]  # row r = c*128+p -> den[p, c]
            out[b, :, h * HD : (h + 1) * HD] = (
                o[j].astype(np.float32) / d[:, None]
            )
    return out
